# revision 9
# baseline (speedup 1.0000x reference)
"""AdaAttN 3D stylizer — distributed Bass kernel for 8 TRN2 NeuronCores.

Sharding: data-parallel over batch (2) x sequence-parallel over content
tokens N (4-way) -> 8 cores. Core c handles batch b=c//4, token slice
[(c%4)*2048, (c%4+1)*2048). Style tokens (M=4096) are fully replicated
per core, so the attention softmax / mean / var reductions over M are
local. The only cross-core reduction is the instance-norm statistics of
q0/c0 over the full N=8192 axis: a single 4KB AllGather per batch group
([[0..3],[4..7]]), launched right after each zipper and fetched only
when the style-side work has covered the latency.

Algebraic fold: with q = Qe inorm(q0) + bq and k = Ke inorm(s0) + bk,
dropping per-query constants (softmax-invariant) gives
    logits[n,m] = (q0*rsq)^T (wg*rss)^T s0  + beta_m
with wg = Ke^T Qe folded on the host, and
    beta = (u*rss)^T s0 + nbq^T g,  u = Ke^T bq,  nbq = -mu_q*rsq.
This removes the qe/ke projections and both attention-path instance
norms; beta rides the exp activation bias. std = exp(0.5*ln(var)) keeps
the whole attention phase inside one ACT table set (no table reloads
against the exp stream).

Compute dtype: bf16 on the PE (fp32 PSUM accumulation), fp32 for all
statistics and the softmax denominators.
"""

import sys
import types

import numpy as np
import ml_dtypes

if "/opt/trn_rl_repo" not in sys.path:
    sys.path.insert(0, "/opt/trn_rl_repo")


def _install_ntff_shim():
    """Make run_bass_kernel_spmd(trace=True) degrade gracefully (or work,
    when the axon profiling lib is present) even if antenv.axon_hooks is
    not importable in this image."""
    try:
        import antenv.axon_hooks  # noqa: F401
        return
    except ImportError:
        pass
    mod = types.ModuleType("antenv.axon_hooks")
    mod._hook = None

    def set_axon_ntff_profile_hook(h):
        mod._hook = h

    def get_axon_ntff_profile_hook():
        return mod._hook

    mod.set_axon_ntff_profile_hook = set_axon_ntff_profile_hook
    mod.get_axon_ntff_profile_hook = get_axon_ntff_profile_hook
    sys.modules["antenv.axon_hooks"] = mod
    try:
        import antenv
        antenv.axon_hooks = mod
    except ImportError:
        pass
    try:
        from trn_agent_boot.trn_boot import _ntff_profile_via_ctypes
        set_axon_ntff_profile_hook(_ntff_profile_via_ctypes("/opt/axon/libaxon_pjrt.so"))
    except Exception:
        pass


_install_ntff_shim()

import concourse.bacc as bacc  # noqa: E402
import concourse.mybir as mybir  # noqa: E402
import concourse.tile as tile  # noqa: E402
from concourse.bass_utils import run_bass_kernel_spmd  # noqa: E402

F32 = mybir.dt.float32
BF16 = mybir.dt.bfloat16
AF = mybir.ActivationFunctionType
ALU = mybir.AluOpType
AXX = mybir.AxisListType.X

BS, C, N = 2, 512, 8192
D, M = 256, 4096
NL = N // 4          # tokens per core
NF = 512             # free-dim chunk (one PSUM bank of f32)
NCH = NL // NF       # chunks per core
MT = M // 128        # style tiles of 128
EPS = 1e-5

# bias-pack column indices ([128, 18] f32)
B_QZ1, B_QZ2, B_VZ1, B_VZ2, B_U, B_VU1, B_VU2 = 0, 2, 4, 6, 8, 12, 14

_nc_cache = {}


def _lrelu_dve(nc, out_ap, in_ap):
    # lrelu(x) = max(0.2*x, x)
    nc.vector.scalar_tensor_tensor(out_ap, in_ap, 0.2, in_ap, ALU.mult, ALU.max)


def _build_nc():
    nc = bacc.Bacc("TRN2", target_bir_lowering=False, debug=False)

    x_d = nc.dram_tensor("x", [C, NL], BF16, kind="ExternalInput")
    sty_d = nc.dram_tensor("sty", [D, M], BF16, kind="ExternalInput")
    wq1_d = nc.dram_tensor("wq1", [C, D], BF16, kind="ExternalInput")
    wq2_d = nc.dram_tensor("wq2", [D, D], BF16, kind="ExternalInput")
    wv1_d = nc.dram_tensor("wv1", [C, D], BF16, kind="ExternalInput")
    wv2_d = nc.dram_tensor("wv2", [D, D], BF16, kind="ExternalInput")
    wg_d = nc.dram_tensor("wg", [D, D], BF16, kind="ExternalInput")
    wse_d = nc.dram_tensor("wse", [D + 1, D], BF16, kind="ExternalInput")
    wu1_d = nc.dram_tensor("wu1", [D, D], BF16, kind="ExternalInput")
    wu2_d = nc.dram_tensor("wu2", [D, C], BF16, kind="ExternalInput")
    bias_d = nc.dram_tensor("bias", [128, 18], F32, kind="ExternalInput")
    eye_d = nc.dram_tensor("eye32", [32, 32], F32, kind="ExternalInput")
    out_d = nc.dram_tensor("out", [C, NL], F32, kind="ExternalOutput")

    with tile.TileContext(nc) as tc:
        with tc.tile_pool(name="wp", bufs=1) as wp, \
             tc.tile_pool(name="bigp", bufs=1) as bigp, \
             tc.tile_pool(name="workp", bufs=2) as workp, \
             tc.tile_pool(name="dramp", bufs=1, space="DRAM") as dp:

            # ---- weights / constants ----
            def wload(name, dram, kparts, nfree):
                t = wp.tile([128, kparts, nfree], BF16, name=name, tag=name)
                for ki in range(kparts):
                    nc.sync.dma_start(t[:, ki, :], dram[ki * 128:(ki + 1) * 128, :])
                return t

            wq1_sb = wload("wq1_sb", wq1_d, 4, 256)
            bias_sb = wp.tile([128, 18], F32, name="bias_sb", tag="bias_sb")
            nc.sync.dma_start(bias_sb[:], bias_d[:])
            # x loaded in column halves so the first zipper chunks can start
            # before the whole 2MB lands
            xb = [bigp.tile([128, NL], BF16, name=f"xb{ci}", tag=f"xb{ci}")
                  for ci in range(4)]
            for half in range(2):
                csl = slice(half * (NL // 2), (half + 1) * (NL // 2))
                for ci in range(4):
                    nc.sync.dma_start(xb[ci][:, csl],
                                      x_d[ci * 128:(ci + 1) * 128, csl])
            wq2_sb = wload("wq2_sb", wq2_d, 2, 256)
            wv1_sb = wload("wv1_sb", wv1_d, 4, 256)
            wv2_sb = wload("wv2_sb", wv2_d, 2, 256)
            styb = []
            for oj in range(2):
                t = bigp.tile([128, M], BF16, name=f"styb{oj}", tag=f"styb{oj}")
                nc.sync.dma_start(t[:], sty_d[oj * 128:(oj + 1) * 128, :])
                styb.append(t)
            wg_sb = wload("wg_sb", wg_d, 2, 256)
            wu1_sb = wload("wu1_sb", wu1_d, 2, 256)
            wu2_sb = wload("wu2_sb", wu2_d, 2, 512)
            wse_sb = wp.tile([128, 3, 256], BF16, name="wse_sb", tag="wse_sb")
            for ki in range(2):
                nc.sync.dma_start(wse_sb[:, ki, :], wse_d[ki * 128:(ki + 1) * 128, :])
            nc.sync.dma_start(wse_sb[0:1, 2, :], wse_d[256:257, :])
            eye_sb = wp.tile([32, 32], F32, name="eye_sb", tag="eye_sb")
            nc.sync.dma_start(eye_sb[:], eye_d[:])
            ones_bf = wp.tile([1, 128], BF16, name="ones_bf", tag="ones_bf")
            nc.vector.memset(ones_bf[:], 1.0)
            onecol_bf = wp.tile([128, 1], BF16, name="onecol_bf", tag="onecol_bf")
            nc.vector.memset(onecol_bf[:], 1.0)
            ones_f32 = wp.tile([1, 128], F32, name="ones_f32", tag="ones_f32")
            nc.vector.memset(ones_f32[:], 1.0)
            eps_sb = wp.tile([128, 1], F32, name="eps_sb", tag="eps_sb")
            nc.vector.memset(eps_sb[:], EPS)

            # startup barrier: absorb cross-core NEFF-launch skew here (PE is
            # loading weights anyway) so the real stats collectives later see
            # aligned peers instead of paying the skew on the critical path
            sync_in = dp.tile([1, 1], F32, name="sync_in")
            sync_out = dp.tile([8, 1], F32, name="sync_out")
            sync_sb = wp.tile([1, 1], F32, name="sync_sb", tag="sync_sb")
            nc.vector.memset(sync_sb[:], 0.0)
            nc.gpsimd.dma_start(sync_in[:], sync_sb[:])
            nc.gpsimd.collective_compute(
                "AllGather", ALU.bypass,
                replica_groups=[[0, 1, 2, 3, 4, 5, 6, 7]],
                ins=[sync_in[:].opt()], outs=[sync_out[:].opt()])

            with tc.tile_pool(name="ppA", bufs=8, space="PSUM") as ppA:
                # ---- phase 1: zipper MLPs + partial stats ----
                parts = bigp.tile([128, 32], F32, name="parts", tag="parts")

                def zipper(w1_sb, w2_sb, b1c, b2c, pfx, sum_base, sq_base):
                    # h = lrelu(w1.T @ x + b1)  -> bf16 [2][128, NL]
                    h = [bigp.tile([128, NL], BF16, name=f"{pfx}h{oj}", tag=f"h1_{oj}")
                         for oj in range(2)]
                    for jn in range(NCH):
                        for oj in range(2):
                            ps = ppA.tile([128, NF], F32, name="psz", tag="mm")
                            for ki in range(4):
                                nc.tensor.matmul(
                                    ps[:], w1_sb[:, ki, oj * 128:(oj + 1) * 128],
                                    xb[ki][:, jn * NF:(jn + 1) * NF],
                                    start=(ki == 0), stop=(ki == 3))
                            hl = workp.tile([128, NF], BF16, name="hl", tag="scr512", bufs=3)
                            nc.scalar.activation(hl[:], ps[:], AF.Identity,
                                                 bias=bias_sb[:, b1c + oj:b1c + oj + 1])
                            _lrelu_dve(nc, h[oj][:, jn * NF:(jn + 1) * NF], hl[:])
                    # o = w2.T @ h + b2 -> bf16 [2][128, NL], plus sum/sumsq partials
                    o = [bigp.tile([128, NL], BF16, name=f"{pfx}o{oj}", tag=f"{pfx}o{oj}")
                         for oj in range(2)]
                    for oj in range(2):
                        for jn in range(NCH):
                            ps = ppA.tile([128, NF], F32, name="psz2", tag="mm")
                            for ki in range(2):
                                nc.tensor.matmul(
                                    ps[:], w2_sb[:, ki, oj * 128:(oj + 1) * 128],
                                    h[ki][:, jn * NF:(jn + 1) * NF],
                                    start=(ki == 0), stop=(ki == 1))
                            nc.scalar.activation(
                                o[oj][:, jn * NF:(jn + 1) * NF], ps[:], AF.Identity,
                                bias=bias_sb[:, b2c + oj:b2c + oj + 1],
                                accum_out=parts[:, sum_base + oj * 4 + jn:sum_base + oj * 4 + jn + 1])
                            sq = workp.tile([128, NF], BF16, name="sq", tag="scr512", bufs=3)
                            nc.vector.scalar_tensor_tensor(
                                sq[:], o[oj][:, jn * NF:(jn + 1) * NF], 0.0,
                                o[oj][:, jn * NF:(jn + 1) * NF], ALU.add, ALU.mult,
                                accum_out=parts[:, sq_base + oj * 4 + jn:sq_base + oj * 4 + jn + 1])
                    return o

                def stats_launch(pfx, base):
                    # reduce partials -> [128, 4] (sum0 sum1 sq0 sq1), AllGather
                    # within the batch group (lower floor than AllReduce).
                    sin = workp.tile([128, 4], F32, name=f"sin_{pfx}", tag=f"sin_{pfx}",
                                     bufs=1)
                    for g in range(4):
                        nc.vector.reduce_sum(sin[:, g:g + 1],
                                             parts[:, base + g * 4:base + (g + 1) * 4],
                                             axis=AXX)
                    # collective staging DMAs go on gpsimd's queue: the
                    # result-fetch DMA blocks on the collective semaphore, and
                    # on the (in-order) sync queue it would stall every later
                    # load behind it.
                    cin = dp.tile([128, 4], F32, name=f"cc_in_{pfx}")
                    cout = dp.tile([4, 128, 4], F32, name=f"cc_out_{pfx}")
                    nc.gpsimd.dma_start(cin[:], sin[:])
                    nc.gpsimd.collective_compute(
                        "AllGather", ALU.bypass,
                        replica_groups=[[0, 1, 2, 3], [4, 5, 6, 7]],
                        ins=[cin[:].opt()], outs=[cout[:].opt()])
                    return cout

                def stats_fetch(pfx, cout):
                    # emitted late: the DVE reduce would otherwise park the
                    # in-order DVE queue on the collective semaphore
                    sg4 = workp.tile([128, 4, 4], F32, name=f"sg4_{pfx}",
                                     tag=f"sg4_{pfx}", bufs=1)
                    nc.gpsimd.dma_start(sg4[:], cout.rearrange("r p c -> p c r"))
                    sg = workp.tile([128, 4], F32, name=f"sg_{pfx}", tag=f"sg_{pfx}",
                                    bufs=1)
                    nc.vector.reduce_sum(sg[:], sg4[:], axis=AXX)
                    return sg

                def norm_consts(sum_ap, sq_ap, n_axis, pfx):
                    # returns (rsig, nbias) with nbias = -mu * rsig
                    mu = workp.tile([128, 1], F32, name=f"{pfx}mu", tag=f"{pfx}mu", bufs=1)
                    nc.scalar.mul(mu[:], sum_ap, 1.0 / n_axis)
                    ex2 = workp.tile([128, 1], F32, name=f"{pfx}ex2", tag=f"{pfx}ex2", bufs=1)
                    nc.scalar.mul(ex2[:], sq_ap, 1.0 / n_axis)
                    nvar = workp.tile([128, 1], F32, name=f"{pfx}nvar", tag=f"{pfx}nvar", bufs=1)
                    # nvar = mu*mu - ex2  (= -var)
                    nc.vector.scalar_tensor_tensor(nvar[:], mu[:], mu[:], ex2[:],
                                                   ALU.mult, ALU.subtract)
                    sig = workp.tile([128, 1], F32, name=f"{pfx}sig", tag=f"{pfx}sig", bufs=1)
                    nc.scalar.activation(sig[:], nvar[:], AF.Sqrt, bias=eps_sb[:, 0:1],
                                         scale=-1.0)
                    rsig = bigp.tile([128, 1], F32, name=f"{pfx}rsig", tag=f"{pfx}rsig")
                    nc.vector.reciprocal(rsig[:], sig[:])
                    nbias = bigp.tile([128, 1], F32, name=f"{pfx}nb", tag=f"{pfx}nb")
                    nc.vector.scalar_tensor_tensor(nbias[:], mu[:], -1.0, rsig[:],
                                                   ALU.mult, ALU.mult)
                    return rsig, nbias

                # ---- both zippers first; their collectives overlap the
                # style-side work below ----
                q0 = zipper(wq1_sb, wq2_sb, B_QZ1, B_QZ2, "q0", 0, 8)
                q_cout = stats_launch("q", 0)
                c0 = zipper(wv1_sb, wv2_sb, B_VZ1, B_VZ2, "c0", 16, 24)
                c_cout = stats_launch("c", 16)

                # ---- phase 2 (collective-independent): style side ----
                ssum = workp.tile([128, 2], F32, name="ssum", tag="ssum", bufs=1)
                ssq = workp.tile([128, 2], F32, name="ssq", tag="ssq", bufs=1)
                for oj in range(2):
                    sqa = workp.tile([128, M], BF16, name=f"sqa{oj}", tag="sqs", bufs=1)
                    nc.scalar.activation(sqa[:], styb[oj][:], AF.Identity,
                                         accum_out=ssum[:, oj:oj + 1])
                    sqb = workp.tile([128, M], BF16, name=f"sqb{oj}", tag="xf", bufs=1)
                    nc.vector.scalar_tensor_tensor(
                        sqb[:], styb[oj][:], 0.0, styb[oj][:], ALU.add, ALU.mult,
                        accum_out=ssq[:, oj:oj + 1])

                sty_rs = []
                for oj in range(2):
                    rs, _ = norm_consts(ssum[:, oj:oj + 1], ssq[:, oj:oj + 1], M, f"sn{oj}")
                    sty_rs.append(rs)

                # s_projT (+bias row) and sv = [sT | sT^2]; the square comes
                # straight out of PSUM on ACT, the copy on DVE
                sv = []
                for mt in range(MT):
                    ps = ppA.tile([128, 256], F32, name="pssv", tag="mm")
                    for ki in range(2):
                        nc.tensor.matmul(ps[:], styb[ki][:, mt * 128:(mt + 1) * 128],
                                         wse_sb[:, ki, :], start=(ki == 0), stop=False)
                    nc.tensor.matmul(ps[:], ones_bf[0:1, :], wse_sb[0:1, 2, :],
                                     start=False, stop=True)
                    t = bigp.tile([128, 512], BF16, name=f"sv{mt}", tag=f"sv{mt}")
                    nc.vector.tensor_copy(t[:, 0:256], ps[:])
                    nc.scalar.square(t[:, 256:512], ps[:])
                    sv.append(t)

                # g = (wg * rss_row)^T s0  (replaces the ke-projection; wg is
                # the host-folded Ke^T Qe with rows indexed by the style dim)
                wg_s = wp.tile([128, 2, 256], BF16, name="wg_s", tag="wg_s")
                for ki in range(2):
                    nc.vector.tensor_scalar_mul(wg_s[:, ki, :], wg_sb[:, ki, :],
                                                sty_rs[ki][:])
                g = []
                for oj in range(2):
                    t = bigp.tile([128, M], BF16, name=f"g{oj}", tag=f"kpb{oj}")
                    for mc in range(M // NF):
                        ps = ppA.tile([128, NF], F32, name="psg", tag="mm")
                        for ki in range(2):
                            nc.tensor.matmul(ps[:], wg_s[:, ki, oj * 128:(oj + 1) * 128],
                                             styb[ki][:, mc * NF:(mc + 1) * NF],
                                             start=(ki == 0), stop=(ki == 1))
                        if mc % 2 == 0:
                            nc.scalar.activation(t[:, mc * NF:(mc + 1) * NF], ps[:],
                                                 AF.Identity)
                        else:
                            nc.vector.tensor_copy(t[:, mc * NF:(mc + 1) * NF], ps[:])
                    g.append(t)

                # ---- phase 3: fetch + apply collective stats ----
                stats_gq = stats_fetch("q", q_cout)
                q_rs, q_nb = [], []
                for oj in range(2):
                    rs, nb = norm_consts(stats_gq[:, oj:oj + 1],
                                         stats_gq[:, 2 + oj:3 + oj], N, f"qn{oj}")
                    q_rs.append(rs)
                    q_nb.append(nb)

                # q0n = q0 * rsq  (mean fold lives in beta); split DVE/Pool
                q0n = []
                for oj in range(2):
                    t = bigp.tile([128, NL], BF16, name=f"q0n{oj}", tag=f"xb{oj}")
                    hl = NL // 2
                    nc.vector.tensor_scalar_mul(t[:, 0:hl], q0[oj][:, 0:hl],
                                                q_rs[oj][:])
                    nc.gpsimd.tensor_scalar_mul(t[:, hl:NL], q0[oj][:, hl:NL],
                                                q_rs[oj][:])
                    q0n.append(t)

                # beta_m = (u*rss)^T s0 + nbq^T g   (bf16 lhsT columns)
                su_bf = wp.tile([128, 4], BF16, name="su_bf", tag="su_bf")
                for oj in range(2):
                    su_f = workp.tile([128, 1], F32, name=f"su{oj}", tag=f"su{oj}", bufs=1)
                    nc.vector.tensor_scalar_mul(su_f[:], bias_sb[:, B_U + oj:B_U + oj + 1],
                                                sty_rs[oj][:])
                    nc.vector.tensor_copy(su_bf[:, oj:oj + 1], su_f[:])
                    nc.vector.tensor_copy(su_bf[:, 2 + oj:3 + oj], q_nb[oj][:])
                # beta psum rows -> [1, NF] stages (ACT/DVE split; DMA cannot
                # read PSUM), 4-row DMAs into brow32, one PE transpose
                brow32 = wp.tile([32, 128], F32, name="brow32", tag="brow32")
                for mc in range(M // NF):
                    ps = ppA.tile([1, NF], F32, name="psb", tag="mm")
                    for oj in range(2):
                        nc.tensor.matmul(ps[:], su_bf[:, oj:oj + 1],
                                         styb[oj][:, mc * NF:(mc + 1) * NF],
                                         start=(oj == 0), stop=False)
                    for oj in range(2):
                        nc.tensor.matmul(ps[:], su_bf[:, 2 + oj:3 + oj],
                                         g[oj][:, mc * NF:(mc + 1) * NF],
                                         start=False, stop=(oj == 1))
                    stg = workp.tile([1, NF], F32, name=f"bstg{mc}", tag="bstg",
                                     bufs=2)
                    if mc % 2 == 0:
                        nc.scalar.copy(stg[:], ps[:])
                    else:
                        nc.vector.tensor_copy(stg[:], ps[:])
                    nc.sync.dma_start(brow32[mc * 4:(mc + 1) * 4, :], stg[:])
                bcp = ppA.tile([128, 32], F32, name="bcp", tag="mm")
                nc.tensor.transpose(bcp[:], brow32[:], eye_sb[:])
                bcol = wp.tile([128, MT], F32, name="bcol", tag="bcol")
                nc.vector.tensor_copy(bcol[:], bcp[:])

                stats_gc = stats_fetch("c", c_cout)
                c_rs, c_nb = [], []
                for oj in range(2):
                    rs, nb = norm_consts(stats_gc[:, oj:oj + 1],
                                         stats_gc[:, 2 + oj:3 + oj], N, f"cn{oj}")
                    c_rs.append(rs)
                    c_nb.append(nb)
                c0n = []
                for oj in range(2):
                    t = bigp.tile([128, NL], BF16, name=f"c0n{oj}", tag=f"xb{2 + oj}")
                    hl = NL // 2
                    nc.gpsimd.tensor_scalar(t[:, 0:hl], c0[oj][:, 0:hl], c_rs[oj][:],
                                            c_nb[oj][:], ALU.mult, ALU.add)
                    nc.vector.tensor_scalar(t[:, hl:NL], c0[oj][:, hl:NL], c_rs[oj][:],
                                            c_nb[oj][:], ALU.mult, ALU.add)
                    c0n.append(t)

            # ---- phase 4: attention + epilogue, software-pipelined across
            # chunks of 512 queries. The S/exp stream runs 2 tiles ahead of
            # the O accumulation and crosses chunk boundaries; each chunk's
            # output MLP is deferred into the next chunk's O-loop so the PE
            # never idles on the epilogue's DVE chain.
            with tc.tile_pool(name="ppB", bufs=1, space="PSUM") as ppB, \
                 tc.tile_pool(name="ep", bufs=2) as ep:
                eS = [[None] * MT for _ in range(NCH)]
                s_next = [0]

                def s_step():
                    gidx = s_next[0]
                    s_next[0] += 1
                    if gidx >= NCH * MT:
                        return
                    jc, mt = divmod(gidx, MT)
                    nsl = slice(jc * NF, (jc + 1) * NF)
                    ps = ppB.tile([128, NF], F32, name=f"pss{gidx}", tag="sT", bufs=3)
                    for ki in range(2):
                        nc.tensor.matmul(ps[:], g[ki][:, mt * 128:(mt + 1) * 128],
                                         q0n[ki][:, nsl], start=(ki == 0), stop=(ki == 1))
                    e = ep.tile([128, NF], BF16, name=f"eS{gidx}", tag="eS", bufs=6)
                    nc.scalar.activation(e[:], ps[:], AF.Exp,
                                         bias=bcol[:, mt:mt + 1])
                    eS[jc][mt] = e

                def epilogue(jc, po, rz, h0, h1):
                    # process columns [h0:h1) of the chunk
                    nsl = slice(jc * NF + h0, jc * NF + h1)
                    hsl = slice(h0, h1)
                    hw = h1 - h0
                    # copy po out of PSUM (split across engines) so the po
                    # slots free without waiting on the epilogue chain
                    osb = []
                    for gi in range(4):
                        t = ep.tile([128, hw], F32, name=f"osb{jc}_{gi}_{h0}",
                                    tag=f"osb{gi}", bufs=1)
                        # Pool cannot read PSUM on TRN2: split ACT/DVE
                        if gi % 2 == 0:
                            nc.scalar.copy(t[:], po[gi][:, hsl])
                        else:
                            nc.vector.tensor_copy(t[:], po[gi][:, hsl])
                        osb.append(t)
                    bzp = ppB.tile([128, hw], F32, name=f"bzp{jc}_{h0}", tag="mlp")
                    nc.tensor.matmul(bzp[:], ones_f32[0:1, :], rz[0:1, hsl])
                    bz = ep.tile([128, hw], F32, name=f"bz{jc}_{h0}", tag="bz", bufs=1)
                    nc.scalar.copy(bz[:], bzp[:])
                    cs = []

                    def etmp(nm, bufs=3):
                        return ep.tile([128, hw], F32, name=f"{nm}{jc}_{h0}",
                                       tag="etmp", bufs=bufs)

                    for oj in range(2):
                        mean = ep.tile([128, hw], F32, name=f"mean{jc}_{oj}_{h0}",
                                       tag="mean", bufs=2)
                        if oj == 0:
                            nc.vector.tensor_mul(mean[:], osb[oj][:], bz[:])
                        else:
                            nc.gpsimd.tensor_mul(mean[:], osb[oj][:], bz[:])
                        es2 = etmp(f"es2_{oj}_")
                        if oj == 0:
                            nc.gpsimd.tensor_mul(es2[:], osb[2 + oj][:], bz[:])
                        else:
                            nc.vector.tensor_mul(es2[:], osb[2 + oj][:], bz[:])
                        msq = etmp(f"msq_{oj}_")
                        nc.scalar.square(msq[:], mean[:])
                        var = etmp(f"var_{oj}_")
                        nc.vector.tensor_sub(var[:], es2[:], msq[:])
                        varp = etmp(f"varp_{oj}_")
                        # floor slightly above 0: ln(0) = -inf would NaN the
                        # table-based exp; sqrt(1e-35) ~ 0 is indistinguishable
                        nc.gpsimd.tensor_scalar_max(varp[:], var[:], 1e-35)
                        # std = exp(0.5*ln(varp)): Ln/Exp share one ACT table
                        # set, so no table reloads against the exp stream
                        lnv = etmp(f"lnv_{oj}_")
                        nc.scalar.activation(lnv[:], varp[:], AF.Ln)
                        std = etmp(f"std_{oj}_")
                        nc.scalar.activation(std[:], lnv[:], AF.Exp, scale=0.5)
                        t1 = etmp(f"t1_{oj}_")
                        nc.vector.tensor_mul(t1[:], c0n[oj][:, nsl], std[:])
                        cst = ep.tile([128, hw], BF16, name=f"cst{jc}_{oj}_{h0}",
                                      tag="cst", bufs=2)
                        nc.vector.tensor_add(cst[:], t1[:], mean[:])
                        cs.append(cst)
                    return cs

                def make_mlp(jc, cs, h0, h1):
                    nsl = slice(jc * NF + h0, jc * NF + h1)
                    hw = h1 - h0
                    hb = []

                    def h_step(oj):
                        ps = ppB.tile([128, hw], F32, name=f"psh{jc}_{oj}_{h0}",
                                      tag="mlp")
                        for ki in range(2):
                            nc.tensor.matmul(ps[:], wu1_sb[:, ki, oj * 128:(oj + 1) * 128],
                                             cs[ki][:], start=(ki == 0), stop=(ki == 1))
                        hl = ep.tile([128, hw], BF16, name=f"hl4{jc}_{oj}_{h0}",
                                     tag="hl4", bufs=2)
                        nc.scalar.activation(hl[:], ps[:], AF.Identity,
                                             bias=bias_sb[:, B_VU1 + oj:B_VU1 + oj + 1])
                        ht = ep.tile([128, hw], BF16, name=f"hb{jc}_{oj}_{h0}",
                                     tag="hb", bufs=2)
                        _lrelu_dve(nc, ht[:], hl[:])
                        hb.append(ht)

                    def o_step(oc):
                        ps = ppB.tile([128, hw], F32, name=f"pso{jc}_{oc}_{h0}",
                                      tag="mlp")
                        for ki in range(2):
                            nc.tensor.matmul(ps[:], wu2_sb[:, ki, oc * 128:(oc + 1) * 128],
                                             hb[ki][:], start=(ki == 0), stop=(ki == 1))
                        of = ep.tile([128, hw], F32, name=f"of{jc}_{oc}_{h0}",
                                     tag="of", bufs=2)
                        nc.scalar.activation(of[:], ps[:], AF.Identity,
                                             bias=bias_sb[:, B_VU2 + oc:B_VU2 + oc + 1])
                        nc.sync.dma_start(out_d[oc * 128:(oc + 1) * 128, nsl], of[:])

                    return ([lambda oj=oj: h_step(oj) for oj in range(2)]
                            + [lambda oc=oc: o_step(oc) for oc in range(4)])

                deferred = {}
                s_step()
                s_step()
                s_step()
                for jc in range(NCH):
                    po = [ppB.tile([128, NF], F32, name=f"po{jc}_{gi}", tag=f"po{gi}")
                          for gi in range(4)]
                    acc_d = acc_p = None
                    for mt in range(MT):
                        s_step()
                        st, sp = (mt == 0), (mt == MT - 1)
                        for gi in range(4):
                            nc.tensor.matmul(po[gi][:], sv[mt][:, gi * 128:(gi + 1) * 128],
                                             eS[jc][mt][:], start=st, stop=sp)
                        # running sum of eS tiles split across DVE (even tiles)
                        # and Pool (odd tiles); combined at the end
                        if mt % 2 == 0:
                            na = ep.tile([128, NF], F32, name=f"zad{jc}_{mt}",
                                         tag="zaccd", bufs=2)
                            if acc_d is None:
                                nc.vector.tensor_copy(na[:], eS[jc][mt][:])
                            else:
                                nc.vector.tensor_add(na[:], acc_d[:], eS[jc][mt][:])
                            acc_d = na
                        else:
                            na = ep.tile([128, NF], F32, name=f"zap{jc}_{mt}",
                                         tag="zaccp", bufs=2)
                            if acc_p is None:
                                nc.gpsimd.tensor_copy(na[:], eS[jc][mt][:])
                            else:
                                nc.gpsimd.tensor_add(na[:], acc_p[:], eS[jc][mt][:])
                            acc_p = na
                        for fn in deferred.pop((jc, mt), []):
                            fn()
                    acc = ep.tile([128, NF], BF16, name=f"zacc{jc}", tag="zacc", bufs=1)
                    nc.vector.tensor_add(acc[:], acc_d[:], acc_p[:])
                    zps = ppB.tile([1, NF], F32, name=f"zps{jc}", tag="mlp")
                    nc.tensor.matmul(zps[:], onecol_bf[:], acc[:])
                    rz = ep.tile([1, NF], F32, name=f"rz{jc}", tag="rz")
                    nc.vector.reciprocal_approx_fast(rz[:], zps[:])
                    if jc + 1 < NCH:
                        cs = epilogue(jc, po, rz, 0, NF)
                        mlp_fns = make_mlp(jc, cs, 0, NF)
                        for idx, fn in enumerate(mlp_fns):
                            deferred.setdefault((jc + 1, 8 + idx * 3), []).append(fn)
                    else:
                        # tail: run in column halves so the PE's MLP matmuls of
                        # half 0 overlap the DVE/ACT epilogue chain of half 1
                        cs0 = epilogue(jc, po, rz, 0, NF // 2)
                        fns0 = make_mlp(jc, cs0, 0, NF // 2)
                        cs1 = epilogue(jc, po, rz, NF // 2, NF)
                        fns1 = make_mlp(jc, cs1, NF // 2, NF)
                        for f0 in fns0:
                            f0()
                        for f1 in fns1:
                            f1()

    nc.compile()
    return nc


def _get_nc():
    if "nc" not in _nc_cache:
        _nc_cache["nc"] = _build_nc()
    return _nc_cache["nc"]


def _prep_inputs(inputs):
    bf = ml_dtypes.bfloat16
    t = lambda a: np.ascontiguousarray(np.asarray(a).T).astype(bf)

    qe_w = np.asarray(inputs["qe_w"], np.float32)
    ke_w = np.asarray(inputs["ke_w"], np.float32)
    qe_b = np.asarray(inputs["qe_b"], np.float32)
    shared = {
        "wq1": t(inputs["qz_w1"]), "wq2": t(inputs["qz_w2"]),
        "wv1": t(inputs["vz_w1"]), "wv2": t(inputs["vz_w2"]),
        # wg rows are style-dim (f), cols content-dim (e):
        # logits = (q0*rsq)^T (wg*rss)^T s0
        "wg": np.ascontiguousarray(ke_w.T @ qe_w).astype(bf),
        "wu1": t(inputs["vu_w1"]), "wu2": t(inputs["vu_w2"]),
        "wse": np.vstack([np.asarray(inputs["se_w"]).T,
                          np.asarray(inputs["se_b"])[None, :]]).astype(bf),
        "eye32": np.eye(32, dtype=np.float32),
    }
    bias = np.zeros((128, 18), np.float32)
    for col, vec in ((B_QZ1, "qz_b1"), (B_QZ2, "qz_b2"), (B_VZ1, "vz_b1"),
                     (B_VZ2, "vz_b2"), (B_VU1, "vu_b1")):
        v = np.asarray(inputs[vec], np.float32)
        bias[:, col] = v[0:128]
        bias[:, col + 1] = v[128:256]
    # u = Ke^T bq for the beta fold (zero when qe_b == 0)
    u = ke_w.T @ qe_b
    bias[:, B_U] = u[0:128]
    bias[:, B_U + 1] = u[128:256]
    v = np.asarray(inputs["vu_b2"], np.float32)
    for i in range(4):
        bias[:, B_VU2 + i] = v[i * 128:(i + 1) * 128]
    shared["bias"] = bias

    x = np.asarray(inputs["feats_in"], np.float32)
    sty = np.asarray(inputs["style_feats"], np.float32)
    in_maps = []
    for c in range(8):
        b, j = divmod(c, 4)
        m = dict(shared)
        m["x"] = np.ascontiguousarray(x[b][:, j * NL:(j + 1) * NL]).astype(bf)
        m["sty"] = np.ascontiguousarray(sty[b]).astype(bf)
        in_maps.append(m)
    return in_maps


def _run(inputs, trace=False):
    nc = _get_nc()
    in_maps = _prep_inputs(inputs)
    res = run_bass_kernel_spmd(nc, in_maps, core_ids=list(range(8)), trace=trace)
    out = np.empty((BS, C, N), np.float32)
    for c in range(8):
        b, j = divmod(c, 4)
        out[b][:, j * NL:(j + 1) * NL] = res.results[c]["out"]
    return out, res


def kernel(**inputs) -> np.ndarray:
    out, _ = _run(inputs, trace=False)
    return out


# revision 10
# speedup vs baseline: 1.2593x; 1.2593x over previous
"""AdaAttN 3D stylizer — distributed Bass kernel for 8 TRN2 NeuronCores.

Sharding: data-parallel over batch (2) x sequence-parallel over content
tokens N (4-way) -> 8 cores. Core c handles batch b=c//4, token slice
[(c%4)*2048, (c%4+1)*2048). Style tokens (M=4096) are fully replicated
per core, so the attention softmax / mean / var reductions over M are
local. The only cross-core reduction is the instance-norm statistics of
q0/c0 over the full N=8192 axis: a single 4KB AllGather per batch group
([[0..3],[4..7]]), launched right after each zipper and fetched only
when the style-side work has covered the latency.

Algebraic fold: with q = Qe inorm(q0) + bq and k = Ke inorm(s0) + bk,
dropping per-query constants (softmax-invariant) gives
    logits[n,m] = (q0*rsq)^T (wg*rss)^T s0  + beta_m
with wg = Ke^T Qe folded on the host, and
    beta = (u*rss)^T s0 + nbq^T g,  u = Ke^T bq,  nbq = -mu_q*rsq.
This removes the qe/ke projections and both attention-path instance
norms; beta rides the exp activation bias. std = exp(0.5*ln(var)) keeps
the whole attention phase inside one ACT table set (no table reloads
against the exp stream).

Compute dtype: bf16 on the PE (fp32 PSUM accumulation), fp32 for all
statistics and the softmax denominators.
"""

import sys
import types

import numpy as np
import ml_dtypes

if "/opt/trn_rl_repo" not in sys.path:
    sys.path.insert(0, "/opt/trn_rl_repo")


def _install_ntff_shim():
    """Make run_bass_kernel_spmd(trace=True) degrade gracefully (or work,
    when the axon profiling lib is present) even if antenv.axon_hooks is
    not importable in this image."""
    try:
        import antenv.axon_hooks  # noqa: F401
        return
    except ImportError:
        pass
    mod = types.ModuleType("antenv.axon_hooks")
    mod._hook = None

    def set_axon_ntff_profile_hook(h):
        mod._hook = h

    def get_axon_ntff_profile_hook():
        return mod._hook

    mod.set_axon_ntff_profile_hook = set_axon_ntff_profile_hook
    mod.get_axon_ntff_profile_hook = get_axon_ntff_profile_hook
    sys.modules["antenv.axon_hooks"] = mod
    try:
        import antenv
        antenv.axon_hooks = mod
    except ImportError:
        pass
    try:
        from trn_agent_boot.trn_boot import _ntff_profile_via_ctypes
        set_axon_ntff_profile_hook(_ntff_profile_via_ctypes("/opt/axon/libaxon_pjrt.so"))
    except Exception:
        pass


_install_ntff_shim()

import concourse.bacc as bacc  # noqa: E402
import concourse.mybir as mybir  # noqa: E402
import concourse.tile as tile  # noqa: E402
from concourse.bass_utils import run_bass_kernel_spmd  # noqa: E402

F32 = mybir.dt.float32
BF16 = mybir.dt.bfloat16
AF = mybir.ActivationFunctionType
ALU = mybir.AluOpType
AXX = mybir.AxisListType.X

BS, C, N = 2, 512, 8192
D, M = 256, 4096
NL = N // 4          # tokens per core
NF = 512             # free-dim chunk (one PSUM bank of f32)
NCH = NL // NF       # chunks per core
MT = M // 128        # style tiles of 128
EPS = 1e-5

# bias-pack column indices ([128, 18] f32)
B_QZ1, B_QZ2, B_VZ1, B_VZ2, B_U, B_VU1, B_VU2 = 0, 2, 4, 6, 8, 12, 14

_nc_cache = {}


def _lrelu_dve(nc, out_ap, in_ap):
    # lrelu(x) = max(0.2*x, x)
    nc.vector.scalar_tensor_tensor(out_ap, in_ap, 0.2, in_ap, ALU.mult, ALU.max)


def _build_nc():
    nc = bacc.Bacc("TRN2", target_bir_lowering=False, debug=False)

    x_d = nc.dram_tensor("x", [C, NL], BF16, kind="ExternalInput")
    sty_d = nc.dram_tensor("sty", [D, M], BF16, kind="ExternalInput")
    wq1_d = nc.dram_tensor("wq1", [C, D], BF16, kind="ExternalInput")
    wq2_d = nc.dram_tensor("wq2", [D, D], BF16, kind="ExternalInput")
    wv1_d = nc.dram_tensor("wv1", [C, D], BF16, kind="ExternalInput")
    wv2_d = nc.dram_tensor("wv2", [D, D], BF16, kind="ExternalInput")
    wg_d = nc.dram_tensor("wg", [D, D], BF16, kind="ExternalInput")
    wse_d = nc.dram_tensor("wse", [D + 1, D], BF16, kind="ExternalInput")
    wu1_d = nc.dram_tensor("wu1", [D, D], BF16, kind="ExternalInput")
    wu2_d = nc.dram_tensor("wu2", [D, C], BF16, kind="ExternalInput")
    bias_d = nc.dram_tensor("bias", [128, 18], F32, kind="ExternalInput")
    eye_d = nc.dram_tensor("eye32", [32, 32], F32, kind="ExternalInput")
    out_d = nc.dram_tensor("out", [C, NL], F32, kind="ExternalOutput")

    with tile.TileContext(nc) as tc:
        with tc.tile_pool(name="wp", bufs=1) as wp, \
             tc.tile_pool(name="bigp", bufs=1) as bigp, \
             tc.tile_pool(name="workp", bufs=2) as workp, \
             tc.tile_pool(name="dramp", bufs=1, space="DRAM") as dp:

            # ---- weights / constants ----
            def wload(name, dram, kparts, nfree):
                t = wp.tile([128, kparts, nfree], BF16, name=name, tag=name)
                for ki in range(kparts):
                    nc.sync.dma_start(t[:, ki, :], dram[ki * 128:(ki + 1) * 128, :])
                return t

            wq1_sb = wload("wq1_sb", wq1_d, 4, 256)
            bias_sb = wp.tile([128, 18], F32, name="bias_sb", tag="bias_sb")
            nc.sync.dma_start(bias_sb[:], bias_d[:])
            # x loaded in column halves so the first zipper chunks can start
            # before the whole 2MB lands
            xb = [bigp.tile([128, NL], BF16, name=f"xb{ci}", tag=f"xb{ci}")
                  for ci in range(4)]
            for half in range(2):
                csl = slice(half * (NL // 2), (half + 1) * (NL // 2))
                for ci in range(4):
                    nc.sync.dma_start(xb[ci][:, csl],
                                      x_d[ci * 128:(ci + 1) * 128, csl])
            wq2_sb = wload("wq2_sb", wq2_d, 2, 256)
            wv1_sb = wload("wv1_sb", wv1_d, 4, 256)
            wv2_sb = wload("wv2_sb", wv2_d, 2, 256)
            styb = []
            for oj in range(2):
                t = bigp.tile([128, M], BF16, name=f"styb{oj}", tag=f"styb{oj}")
                nc.sync.dma_start(t[:], sty_d[oj * 128:(oj + 1) * 128, :])
                styb.append(t)
            wg_sb = wload("wg_sb", wg_d, 2, 256)
            wu1_sb = wload("wu1_sb", wu1_d, 2, 256)
            wu2_sb = wload("wu2_sb", wu2_d, 2, 512)
            wse_sb = wp.tile([128, 3, 256], BF16, name="wse_sb", tag="wse_sb")
            for ki in range(2):
                nc.sync.dma_start(wse_sb[:, ki, :], wse_d[ki * 128:(ki + 1) * 128, :])
            nc.sync.dma_start(wse_sb[0:1, 2, :], wse_d[256:257, :])
            eye_sb = wp.tile([32, 32], F32, name="eye_sb", tag="eye_sb")
            nc.sync.dma_start(eye_sb[:], eye_d[:])
            ones_bf = wp.tile([1, 128], BF16, name="ones_bf", tag="ones_bf")
            nc.vector.memset(ones_bf[:], 1.0)
            onecol_bf = wp.tile([128, 1], BF16, name="onecol_bf", tag="onecol_bf")
            nc.vector.memset(onecol_bf[:], 1.0)
            ones_f32 = wp.tile([1, 128], F32, name="ones_f32", tag="ones_f32")
            nc.vector.memset(ones_f32[:], 1.0)
            eps_sb = wp.tile([128, 1], F32, name="eps_sb", tag="eps_sb")
            nc.vector.memset(eps_sb[:], EPS)

            # startup barrier: absorb cross-core NEFF-launch skew here (PE is
            # loading weights anyway) so the real stats collectives later see
            # aligned peers instead of paying the skew on the critical path
            sync_in = dp.tile([1, 1], F32, name="sync_in")
            sync_out = dp.tile([8, 1], F32, name="sync_out")
            sync_sb = wp.tile([1, 1], F32, name="sync_sb", tag="sync_sb")
            nc.vector.memset(sync_sb[:], 0.0)
            nc.gpsimd.dma_start(sync_in[:], sync_sb[:])
            nc.gpsimd.collective_compute(
                "AllGather", ALU.bypass,
                replica_groups=[[0, 1, 2, 3, 4, 5, 6, 7]],
                ins=[sync_in[:].opt()], outs=[sync_out[:].opt()])

            with tc.tile_pool(name="ppA", bufs=8, space="PSUM") as ppA:
                # ---- phase 1: zipper MLPs + partial stats ----
                parts = bigp.tile([128, 32], F32, name="parts", tag="parts")

                def zipper(w1_sb, w2_sb, b1c, b2c, pfx, sum_base, sq_base):
                    # h = lrelu(w1.T @ x + b1)  -> bf16 [2][128, NL]
                    h = [bigp.tile([128, NL], BF16, name=f"{pfx}h{oj}", tag=f"h1_{oj}")
                         for oj in range(2)]
                    for jn in range(NCH):
                        for oj in range(2):
                            ps = ppA.tile([128, NF], F32, name="psz", tag="mm")
                            for ki in range(4):
                                nc.tensor.matmul(
                                    ps[:], w1_sb[:, ki, oj * 128:(oj + 1) * 128],
                                    xb[ki][:, jn * NF:(jn + 1) * NF],
                                    start=(ki == 0), stop=(ki == 3))
                            hl = workp.tile([128, NF], BF16, name="hl", tag="scr512", bufs=3)
                            nc.scalar.activation(hl[:], ps[:], AF.Identity,
                                                 bias=bias_sb[:, b1c + oj:b1c + oj + 1])
                            _lrelu_dve(nc, h[oj][:, jn * NF:(jn + 1) * NF], hl[:])
                    # o = w2.T @ h + b2 -> bf16 [2][128, NL], plus sum/sumsq partials
                    o = [bigp.tile([128, NL], BF16, name=f"{pfx}o{oj}", tag=f"{pfx}o{oj}")
                         for oj in range(2)]
                    for oj in range(2):
                        for jn in range(NCH):
                            ps = ppA.tile([128, NF], F32, name="psz2", tag="mm")
                            for ki in range(2):
                                nc.tensor.matmul(
                                    ps[:], w2_sb[:, ki, oj * 128:(oj + 1) * 128],
                                    h[ki][:, jn * NF:(jn + 1) * NF],
                                    start=(ki == 0), stop=(ki == 1))
                            nc.scalar.activation(
                                o[oj][:, jn * NF:(jn + 1) * NF], ps[:], AF.Identity,
                                bias=bias_sb[:, b2c + oj:b2c + oj + 1],
                                accum_out=parts[:, sum_base + oj * 4 + jn:sum_base + oj * 4 + jn + 1])
                            sq = workp.tile([128, NF], BF16, name="sq", tag="scr512", bufs=3)
                            nc.vector.scalar_tensor_tensor(
                                sq[:], o[oj][:, jn * NF:(jn + 1) * NF], 0.0,
                                o[oj][:, jn * NF:(jn + 1) * NF], ALU.add, ALU.mult,
                                accum_out=parts[:, sq_base + oj * 4 + jn:sq_base + oj * 4 + jn + 1])
                    return o

                def stats_launch(pfx, base):
                    # reduce partials -> [128, 4] (sum0 sum1 sq0 sq1), AllGather
                    # within the batch group (lower floor than AllReduce).
                    sin = workp.tile([128, 4], F32, name=f"sin_{pfx}", tag=f"sin_{pfx}",
                                     bufs=1)
                    for g in range(4):
                        nc.vector.reduce_sum(sin[:, g:g + 1],
                                             parts[:, base + g * 4:base + (g + 1) * 4],
                                             axis=AXX)
                    # collective staging DMAs go on gpsimd's queue: the
                    # result-fetch DMA blocks on the collective semaphore, and
                    # on the (in-order) sync queue it would stall every later
                    # load behind it.
                    cin = dp.tile([128, 4], F32, name=f"cc_in_{pfx}")
                    cout = dp.tile([4, 128, 4], F32, name=f"cc_out_{pfx}")
                    nc.gpsimd.dma_start(cin[:], sin[:])
                    nc.gpsimd.collective_compute(
                        "AllGather", ALU.bypass,
                        replica_groups=[[0, 1, 2, 3], [4, 5, 6, 7]],
                        ins=[cin[:].opt()], outs=[cout[:].opt()])
                    return cout

                def stats_fetch(pfx, cout):
                    # emitted late: the DVE reduce would otherwise park the
                    # in-order DVE queue on the collective semaphore
                    sg4 = workp.tile([128, 4, 4], F32, name=f"sg4_{pfx}",
                                     tag=f"sg4_{pfx}", bufs=1)
                    nc.gpsimd.dma_start(sg4[:], cout.rearrange("r p c -> p c r"))
                    sg = workp.tile([128, 4], F32, name=f"sg_{pfx}", tag=f"sg_{pfx}",
                                    bufs=1)
                    nc.vector.reduce_sum(sg[:], sg4[:], axis=AXX)
                    return sg

                def norm_consts(sum_ap, sq_ap, n_axis, pfx):
                    # returns (rsig, nbias) with nbias = -mu * rsig
                    mu = workp.tile([128, 1], F32, name=f"{pfx}mu", tag=f"{pfx}mu", bufs=1)
                    nc.scalar.mul(mu[:], sum_ap, 1.0 / n_axis)
                    ex2 = workp.tile([128, 1], F32, name=f"{pfx}ex2", tag=f"{pfx}ex2", bufs=1)
                    nc.scalar.mul(ex2[:], sq_ap, 1.0 / n_axis)
                    nvar = workp.tile([128, 1], F32, name=f"{pfx}nvar", tag=f"{pfx}nvar", bufs=1)
                    # nvar = mu*mu - ex2  (= -var)
                    nc.vector.scalar_tensor_tensor(nvar[:], mu[:], mu[:], ex2[:],
                                                   ALU.mult, ALU.subtract)
                    sig = workp.tile([128, 1], F32, name=f"{pfx}sig", tag=f"{pfx}sig", bufs=1)
                    nc.scalar.activation(sig[:], nvar[:], AF.Sqrt, bias=eps_sb[:, 0:1],
                                         scale=-1.0)
                    rsig = bigp.tile([128, 1], F32, name=f"{pfx}rsig", tag=f"{pfx}rsig")
                    nc.vector.reciprocal(rsig[:], sig[:])
                    nbias = bigp.tile([128, 1], F32, name=f"{pfx}nb", tag=f"{pfx}nb")
                    nc.vector.scalar_tensor_tensor(nbias[:], mu[:], -1.0, rsig[:],
                                                   ALU.mult, ALU.mult)
                    return rsig, nbias

                # ---- both zippers first; their collectives overlap the
                # style-side work below ----
                q0 = zipper(wq1_sb, wq2_sb, B_QZ1, B_QZ2, "q0", 0, 8)
                q_cout = stats_launch("q", 0)
                c0 = zipper(wv1_sb, wv2_sb, B_VZ1, B_VZ2, "c0", 16, 24)
                c_cout = stats_launch("c", 16)

                # ---- phase 2 (collective-independent): style side ----
                ssum = workp.tile([128, 2], F32, name="ssum", tag="ssum", bufs=1)
                ssq = workp.tile([128, 2], F32, name="ssq", tag="ssq", bufs=1)
                for oj in range(2):
                    sqa = workp.tile([128, M], BF16, name=f"sqa{oj}", tag="sqs", bufs=1)
                    nc.scalar.activation(sqa[:], styb[oj][:], AF.Identity,
                                         accum_out=ssum[:, oj:oj + 1])
                    sqb = workp.tile([128, M], BF16, name=f"sqb{oj}", tag="xf", bufs=1)
                    nc.vector.scalar_tensor_tensor(
                        sqb[:], styb[oj][:], 0.0, styb[oj][:], ALU.add, ALU.mult,
                        accum_out=ssq[:, oj:oj + 1])

                sty_rs = []
                for oj in range(2):
                    rs, _ = norm_consts(ssum[:, oj:oj + 1], ssq[:, oj:oj + 1], M, f"sn{oj}")
                    sty_rs.append(rs)

                # s_projT (+bias row) and sv = [sT | sT^2]; the square comes
                # straight out of PSUM on ACT, the copy on DVE
                sv = []
                for mt in range(MT):
                    ps = ppA.tile([128, 256], F32, name="pssv", tag="mm")
                    for ki in range(2):
                        nc.tensor.matmul(ps[:], styb[ki][:, mt * 128:(mt + 1) * 128],
                                         wse_sb[:, ki, :], start=(ki == 0), stop=False)
                    nc.tensor.matmul(ps[:], ones_bf[0:1, :], wse_sb[0:1, 2, :],
                                     start=False, stop=True)
                    t = bigp.tile([128, 512], BF16, name=f"sv{mt}", tag=f"sv{mt}")
                    nc.vector.tensor_copy(t[:, 0:256], ps[:])
                    nc.scalar.square(t[:, 256:512], ps[:])
                    sv.append(t)

                # g = (wg * rss_row)^T s0  (replaces the ke-projection; wg is
                # the host-folded Ke^T Qe with rows indexed by the style dim)
                wg_s = wp.tile([128, 2, 256], BF16, name="wg_s", tag="wg_s")
                for ki in range(2):
                    nc.vector.tensor_scalar_mul(wg_s[:, ki, :], wg_sb[:, ki, :],
                                                sty_rs[ki][:])
                g = []
                for oj in range(2):
                    t = bigp.tile([128, M], BF16, name=f"g{oj}", tag=f"kpb{oj}")
                    for mc in range(M // NF):
                        ps = ppA.tile([128, NF], F32, name="psg", tag="mm")
                        for ki in range(2):
                            nc.tensor.matmul(ps[:], wg_s[:, ki, oj * 128:(oj + 1) * 128],
                                             styb[ki][:, mc * NF:(mc + 1) * NF],
                                             start=(ki == 0), stop=(ki == 1))
                        if mc % 2 == 0:
                            nc.scalar.activation(t[:, mc * NF:(mc + 1) * NF], ps[:],
                                                 AF.Identity)
                        else:
                            nc.vector.tensor_copy(t[:, mc * NF:(mc + 1) * NF], ps[:])
                    g.append(t)

                # ---- phase 3: fetch + apply collective stats ----
                stats_gq = stats_fetch("q", q_cout)
                q_rs, q_nb = [], []
                for oj in range(2):
                    rs, nb = norm_consts(stats_gq[:, oj:oj + 1],
                                         stats_gq[:, 2 + oj:3 + oj], N, f"qn{oj}")
                    q_rs.append(rs)
                    q_nb.append(nb)

                # q0n = q0 * rsq  (mean fold lives in beta); split DVE/Pool
                q0n = []
                for oj in range(2):
                    t = bigp.tile([128, NL], BF16, name=f"q0n{oj}", tag=f"xb{oj}")
                    nc.vector.tensor_scalar_mul(t[:], q0[oj][:], q_rs[oj][:])
                    q0n.append(t)

                # beta_m = (u*rss)^T s0 + nbq^T g   (bf16 lhsT columns)
                su_bf = wp.tile([128, 4], BF16, name="su_bf", tag="su_bf")
                for oj in range(2):
                    su_f = workp.tile([128, 1], F32, name=f"su{oj}", tag=f"su{oj}", bufs=1)
                    nc.vector.tensor_scalar_mul(su_f[:], bias_sb[:, B_U + oj:B_U + oj + 1],
                                                sty_rs[oj][:])
                    nc.vector.tensor_copy(su_bf[:, oj:oj + 1], su_f[:])
                    nc.vector.tensor_copy(su_bf[:, 2 + oj:3 + oj], q_nb[oj][:])
                # beta psum rows -> [1, NF] stages (ACT/DVE split; DMA cannot
                # read PSUM), 4-row DMAs into brow32, one PE transpose
                brow32 = wp.tile([32, 128], F32, name="brow32", tag="brow32")
                for mc in range(M // NF):
                    ps = ppA.tile([1, NF], F32, name="psb", tag="mm")
                    for oj in range(2):
                        nc.tensor.matmul(ps[:], su_bf[:, oj:oj + 1],
                                         styb[oj][:, mc * NF:(mc + 1) * NF],
                                         start=(oj == 0), stop=False)
                    for oj in range(2):
                        nc.tensor.matmul(ps[:], su_bf[:, 2 + oj:3 + oj],
                                         g[oj][:, mc * NF:(mc + 1) * NF],
                                         start=False, stop=(oj == 1))
                    stg = workp.tile([1, NF], F32, name=f"bstg{mc}", tag="bstg",
                                     bufs=2)
                    if mc % 2 == 0:
                        nc.scalar.copy(stg[:], ps[:])
                    else:
                        nc.vector.tensor_copy(stg[:], ps[:])
                    nc.sync.dma_start(brow32[mc * 4:(mc + 1) * 4, :], stg[:])
                bcp = ppA.tile([128, 32], F32, name="bcp", tag="mm")
                nc.tensor.transpose(bcp[:], brow32[:], eye_sb[:])
                bcol = wp.tile([128, MT], F32, name="bcol", tag="bcol")
                nc.vector.tensor_copy(bcol[:], bcp[:])

                stats_gc = stats_fetch("c", c_cout)
                c_rs, c_nb = [], []
                for oj in range(2):
                    rs, nb = norm_consts(stats_gc[:, oj:oj + 1],
                                         stats_gc[:, 2 + oj:3 + oj], N, f"cn{oj}")
                    c_rs.append(rs)
                    c_nb.append(nb)
                c0n = []
                for oj in range(2):
                    t = bigp.tile([128, NL], BF16, name=f"c0n{oj}", tag=f"xb{2 + oj}")
                    nc.vector.tensor_scalar(t[:], c0[oj][:], c_rs[oj][:],
                                            c_nb[oj][:], ALU.mult, ALU.add)
                    c0n.append(t)

            # ---- phase 4: attention + epilogue, software-pipelined across
            # chunks of 512 queries. The S/exp stream runs 2 tiles ahead of
            # the O accumulation and crosses chunk boundaries; each chunk's
            # output MLP is deferred into the next chunk's O-loop so the PE
            # never idles on the epilogue's DVE chain.
            with tc.tile_pool(name="ppB", bufs=1, space="PSUM") as ppB, \
                 tc.tile_pool(name="ep", bufs=2) as ep:
                eS = [[None] * MT for _ in range(NCH)]
                s_next = [0]

                def s_step():
                    gidx = s_next[0]
                    s_next[0] += 1
                    if gidx >= NCH * MT:
                        return
                    jc, mt = divmod(gidx, MT)
                    nsl = slice(jc * NF, (jc + 1) * NF)
                    ps = ppB.tile([128, NF], F32, name=f"pss{gidx}", tag="sT", bufs=3)
                    for ki in range(2):
                        nc.tensor.matmul(ps[:], g[ki][:, mt * 128:(mt + 1) * 128],
                                         q0n[ki][:, nsl], start=(ki == 0), stop=(ki == 1))
                    e = ep.tile([128, NF], BF16, name=f"eS{gidx}", tag="eS", bufs=6)
                    nc.scalar.activation(e[:], ps[:], AF.Exp,
                                         bias=bcol[:, mt:mt + 1])
                    eS[jc][mt] = e

                def epilogue(jc, po, rz, h0, h1):
                    # process columns [h0:h1) of the chunk
                    nsl = slice(jc * NF + h0, jc * NF + h1)
                    hsl = slice(h0, h1)
                    hw = h1 - h0
                    # copy po out of PSUM (split across engines) so the po
                    # slots free without waiting on the epilogue chain
                    osb = []
                    for gi in range(4):
                        t = ep.tile([128, hw], F32, name=f"osb{jc}_{gi}_{h0}",
                                    tag=f"osb{gi}", bufs=1)
                        # Pool cannot read PSUM on TRN2: split ACT/DVE
                        if gi < 2:
                            nc.scalar.copy(t[:], po[gi][:, hsl])
                        else:
                            nc.vector.tensor_copy(t[:], po[gi][:, hsl])
                        osb.append(t)
                    bzp = ppB.tile([128, hw], F32, name=f"bzp{jc}_{h0}", tag="mlp")
                    nc.tensor.matmul(bzp[:], ones_f32[0:1, :], rz[0:1, hsl])
                    bz = ep.tile([128, hw], F32, name=f"bz{jc}_{h0}", tag="bz", bufs=1)
                    nc.scalar.copy(bz[:], bzp[:])
                    cs = []

                    def etmp(nm, bufs=3):
                        return ep.tile([128, hw], F32, name=f"{nm}{jc}_{h0}",
                                       tag="etmp", bufs=bufs)

                    for oj in range(2):
                        mean = ep.tile([128, hw], F32, name=f"mean{jc}_{oj}_{h0}",
                                       tag="mean", bufs=2)
                        nc.vector.tensor_mul(mean[:], osb[oj][:], bz[:])
                        es2 = etmp(f"es2_{oj}_")
                        nc.vector.tensor_mul(es2[:], osb[2 + oj][:], bz[:])
                        msq = etmp(f"msq_{oj}_")
                        nc.scalar.square(msq[:], mean[:])
                        var = etmp(f"var_{oj}_")
                        nc.vector.tensor_sub(var[:], es2[:], msq[:])
                        varp = etmp(f"varp_{oj}_")
                        nc.vector.tensor_scalar_max(varp[:], var[:], 0.0)
                        std = etmp(f"std_{oj}_")
                        nc.scalar.activation(std[:], varp[:], AF.Sqrt)
                        t1 = etmp(f"t1_{oj}_")
                        nc.vector.tensor_mul(t1[:], c0n[oj][:, nsl], std[:])
                        cst = ep.tile([128, hw], BF16, name=f"cst{jc}_{oj}_{h0}",
                                      tag="cst", bufs=2)
                        nc.vector.tensor_add(cst[:], t1[:], mean[:])
                        cs.append(cst)
                    return cs

                def make_mlp(jc, cs, h0, h1):
                    nsl = slice(jc * NF + h0, jc * NF + h1)
                    hw = h1 - h0
                    hb = []

                    def h_step(oj):
                        ps = ppB.tile([128, hw], F32, name=f"psh{jc}_{oj}_{h0}",
                                      tag="mlp")
                        for ki in range(2):
                            nc.tensor.matmul(ps[:], wu1_sb[:, ki, oj * 128:(oj + 1) * 128],
                                             cs[ki][:], start=(ki == 0), stop=(ki == 1))
                        hl = ep.tile([128, hw], BF16, name=f"hl4{jc}_{oj}_{h0}",
                                     tag="hl4", bufs=2)
                        nc.scalar.activation(hl[:], ps[:], AF.Identity,
                                             bias=bias_sb[:, B_VU1 + oj:B_VU1 + oj + 1])
                        ht = ep.tile([128, hw], BF16, name=f"hb{jc}_{oj}_{h0}",
                                     tag="hb", bufs=2)
                        _lrelu_dve(nc, ht[:], hl[:])
                        hb.append(ht)

                    def o_step(oc):
                        ps = ppB.tile([128, hw], F32, name=f"pso{jc}_{oc}_{h0}",
                                      tag="mlp")
                        for ki in range(2):
                            nc.tensor.matmul(ps[:], wu2_sb[:, ki, oc * 128:(oc + 1) * 128],
                                             hb[ki][:], start=(ki == 0), stop=(ki == 1))
                        of = ep.tile([128, hw], F32, name=f"of{jc}_{oc}_{h0}",
                                     tag="of", bufs=2)
                        nc.scalar.activation(of[:], ps[:], AF.Identity,
                                             bias=bias_sb[:, B_VU2 + oc:B_VU2 + oc + 1])
                        nc.sync.dma_start(out_d[oc * 128:(oc + 1) * 128, nsl], of[:])

                    return ([lambda oj=oj: h_step(oj) for oj in range(2)]
                            + [lambda oc=oc: o_step(oc) for oc in range(4)])

                deferred = {}
                s_step()
                s_step()
                s_step()
                for jc in range(NCH):
                    po = [ppB.tile([128, NF], F32, name=f"po{jc}_{gi}", tag=f"po{gi}")
                          for gi in range(4)]
                    acc = None
                    for mt in range(MT):
                        s_step()
                        st, sp = (mt == 0), (mt == MT - 1)
                        for gi in range(4):
                            nc.tensor.matmul(po[gi][:], sv[mt][:, gi * 128:(gi + 1) * 128],
                                             eS[jc][mt][:], start=st, stop=sp)
                        # running sum of eS tiles on DVE (Z partial sums);
                        # last add lands in bf16 for the cheap final contraction
                        dt = BF16 if sp else F32
                        na = ep.tile([128, NF], dt, name=f"za{jc}_{mt}", tag="zacc",
                                     bufs=2)
                        if acc is None:
                            nc.vector.tensor_copy(na[:], eS[jc][mt][:])
                        else:
                            nc.vector.tensor_add(na[:], acc[:], eS[jc][mt][:])
                        acc = na
                        for fn in deferred.pop((jc, mt), []):
                            fn()
                    zps = ppB.tile([1, NF], F32, name=f"zps{jc}", tag="mlp")
                    nc.tensor.matmul(zps[:], onecol_bf[:], acc[:])
                    rz = ep.tile([1, NF], F32, name=f"rz{jc}", tag="rz")
                    nc.vector.reciprocal_approx_fast(rz[:], zps[:])
                    if jc + 1 < NCH:
                        cs = epilogue(jc, po, rz, 0, NF)
                        mlp_fns = make_mlp(jc, cs, 0, NF)
                        for idx, fn in enumerate(mlp_fns):
                            deferred.setdefault((jc + 1, 8 + idx * 3), []).append(fn)
                    else:
                        # tail: run in column halves so the PE's MLP matmuls of
                        # half 0 overlap the DVE/ACT epilogue chain of half 1
                        cs0 = epilogue(jc, po, rz, 0, NF // 2)
                        fns0 = make_mlp(jc, cs0, 0, NF // 2)
                        cs1 = epilogue(jc, po, rz, NF // 2, NF)
                        fns1 = make_mlp(jc, cs1, NF // 2, NF)
                        for f0 in fns0:
                            f0()
                        for f1 in fns1:
                            f1()

    nc.compile()
    return nc


def _get_nc():
    if "nc" not in _nc_cache:
        _nc_cache["nc"] = _build_nc()
    return _nc_cache["nc"]


def _prep_inputs(inputs):
    bf = ml_dtypes.bfloat16
    t = lambda a: np.ascontiguousarray(np.asarray(a).T).astype(bf)

    qe_w = np.asarray(inputs["qe_w"], np.float32)
    ke_w = np.asarray(inputs["ke_w"], np.float32)
    qe_b = np.asarray(inputs["qe_b"], np.float32)
    shared = {
        "wq1": t(inputs["qz_w1"]), "wq2": t(inputs["qz_w2"]),
        "wv1": t(inputs["vz_w1"]), "wv2": t(inputs["vz_w2"]),
        # wg rows are style-dim (f), cols content-dim (e):
        # logits = (q0*rsq)^T (wg*rss)^T s0
        "wg": np.ascontiguousarray(ke_w.T @ qe_w).astype(bf),
        "wu1": t(inputs["vu_w1"]), "wu2": t(inputs["vu_w2"]),
        "wse": np.vstack([np.asarray(inputs["se_w"]).T,
                          np.asarray(inputs["se_b"])[None, :]]).astype(bf),
        "eye32": np.eye(32, dtype=np.float32),
    }
    bias = np.zeros((128, 18), np.float32)
    for col, vec in ((B_QZ1, "qz_b1"), (B_QZ2, "qz_b2"), (B_VZ1, "vz_b1"),
                     (B_VZ2, "vz_b2"), (B_VU1, "vu_b1")):
        v = np.asarray(inputs[vec], np.float32)
        bias[:, col] = v[0:128]
        bias[:, col + 1] = v[128:256]
    # u = Ke^T bq for the beta fold (zero when qe_b == 0)
    u = ke_w.T @ qe_b
    bias[:, B_U] = u[0:128]
    bias[:, B_U + 1] = u[128:256]
    v = np.asarray(inputs["vu_b2"], np.float32)
    for i in range(4):
        bias[:, B_VU2 + i] = v[i * 128:(i + 1) * 128]
    shared["bias"] = bias

    x = np.asarray(inputs["feats_in"], np.float32)
    sty = np.asarray(inputs["style_feats"], np.float32)
    in_maps = []
    for c in range(8):
        b, j = divmod(c, 4)
        m = dict(shared)
        m["x"] = np.ascontiguousarray(x[b][:, j * NL:(j + 1) * NL]).astype(bf)
        m["sty"] = np.ascontiguousarray(sty[b]).astype(bf)
        in_maps.append(m)
    return in_maps


def _run(inputs, trace=False):
    nc = _get_nc()
    in_maps = _prep_inputs(inputs)
    res = run_bass_kernel_spmd(nc, in_maps, core_ids=list(range(8)), trace=trace)
    out = np.empty((BS, C, N), np.float32)
    for c in range(8):
        b, j = divmod(c, 4)
        out[b][:, j * NL:(j + 1) * NL] = res.results[c]["out"]
    return out, res


def kernel(**inputs) -> np.ndarray:
    out, _ = _run(inputs, trace=False)
    return out
